# revision 28
# baseline (speedup 1.0000x reference)
"""GAT layer (dense formulation) on 8 Trainium2 NeuronCores.

Computation (N=4096 nodes, IN_F=512, OUT_F=64, HEADS=4):
    h = (x @ W).reshape(N, H, F)
    s = h . a_src ; t = h . a_dst            (per node, per head)
    e[i,j,k] = leaky_relu(s[i,k] + t[j,k])   masked by adj[i,j]
    attn = softmax_j(e) ; out = attn @ h

Sharding: output rows i (nodes) are sharded 512/core across 8 cores.
Each core computes the full h = x @ W redundantly (cheap), then handles
its own 512 i-rows: logits laid out [j=partitions, i=free] so that the
softmax contraction over j runs on the PE as  [h_k | 1].T @ at_tile,
with the ones-column producing the softmax denominator for free.

V4 factorization — no per-element exp at all. Using
    lrelu(v) = 0.2 v + 0.8 relu(v),   v = s_i + t_j:
    exp(lrelu(v)) = e^{0.2 s} * e^{0.2 t} * max(e^{0.8 v}, 1)
The e^{0.2 s_i} factor is constant along the softmax axis j, so it
cancels between numerator and denominator and is simply dropped. With a
global shift e^{-c} (also cancels) the attention surrogate is
    at[j,i] = max( e^{0.8 s_i} * e^{t_j - c},  e^{0.2 t_j - c} ) * m01
= ONE tensor_scalar (mult,max — 4x bf16) + ONE tensor_tensor mult
(2x bf16) per head-tile on the DVE, per-partition exp scalars coming
from a single tiny [128, 8] ACT exp per tile. The t and 0.2t columns
are produced by the h-matmul itself via extra folded W columns.
"""

import os

import numpy as np
import ml_dtypes

import concourse.bass as bass
import concourse.mybir as mybir
import concourse.tile as tile
from concourse import bacc, bass_utils
from concourse._compat import get_trn_type
from concourse.alu_op_type import AluOpType

# ---------------------------------------------------------------- constants
N = 4096
IN_F = 512
OUT_F = 64
HEADS = 4
ALPHA = 0.2
NCORES = 8
SHARD = N // NCORES            # 512 output rows per core
NT = N // 128                  # 32 j-tiles (and n-tiles)
KC = IN_F // 128               # 4 contraction chunks
# W_ext = [W(256) | Ws(4) | Wt(4) | 0.2*Wt(4)]
WCOLS = HEADS * OUT_F + 3 * HEADS   # 268
TCOL = HEADS * OUT_F + HEADS        # 260: start of the [t | 0.2t] block
HB = OUT_F + 1                 # 65 = per-head [h_k | ones] weight block
DELAY = 2                      # attn matmuls trail h-compute by this many tiles
CSHIFT = 3.0                   # global exp shift (cancels in softmax)

F32 = mybir.dt.float32
BF16 = mybir.dt.bfloat16

# ------------------------------------------------------------- bass program
_PROGRAM = None


def _build_program():
    """One SPMD program; per-core behavior differs only through input data."""
    global _PROGRAM
    if _PROGRAM is not None:
        return _PROGRAM

    nc = bacc.Bacc(get_trn_type() or "TRN2", target_bir_lowering=False)
    act = mybir.ActivationFunctionType

    # x^T, full: xT[f, n] = x[n, f], bf16
    xt_d = nc.dram_tensor("xT", [IN_F, N], BF16, kind="ExternalInput")
    # x-shard transposed: xsT[f, i] = x[shard_start + i, f], bf16
    xs_d = nc.dram_tensor("xsT", [IN_F, SHARD], BF16, kind="ExternalInput")
    w_d = nc.dram_tensor("wext", [IN_F, WCOLS], BF16, kind="ExternalInput")
    # multiplicative mask {0,1}, pre-tiled on host: m01[b, p, q*i] =
    #   (adj.T[b*512 + q*128 + p, shard_i] != 0)   (bf16; 4KB DMA lines)
    m_d = nc.dram_tensor("maskTb", [NT // 4, 128, 4 * SHARD], BF16,
                         kind="ExternalInput")
    out_d = nc.dram_tensor("out", [SHARD, HEADS * OUT_F], F32, kind="ExternalOutput")

    with tile.TileContext(nc) as tc:
        with (
            tc.tile_pool(name="const", bufs=1) as cp,
            tc.tile_pool(name="hpool", bufs=1) as hp,
            tc.tile_pool(name="mpool", bufs=1) as mp,
            tc.tile_pool(name="work", bufs=6) as wp,
            tc.tile_pool(name="endp", bufs=2) as ep,
            tc.tile_pool(name="ps", bufs=2, space="PSUM") as psp,
            tc.tile_pool(name="ph", bufs=2, space="PSUM") as php,
            tc.tile_pool(name="psacc", bufs=1, space="PSUM") as psa,
        ):
            # ---------------- phase A: constants in
            xst = []
            for k in range(KC):
                xs_t = cp.tile([128, SHARD], BF16, name=f"xst{k}", tag=f"xst{k}")
                nc.sync.dma_start(xs_t, xs_d[k * 128 : (k + 1) * 128, :])
                xst.append(xs_t)
            wst = []
            for k in range(KC):
                ws_t = cp.tile([128, 3 * HEADS], BF16, name=f"wst{k}", tag=f"wst{k}")
                nc.sync.dma_start(ws_t, w_d[k * 128 : (k + 1) * 128, HEADS * OUT_F :])
                wst.append(ws_t)
            wsb = []
            for k in range(KC):
                w_t = cp.tile([128, WCOLS], BF16, name=f"wsb{k}", tag=f"wsb{k}")
                nc.sync.dma_start(w_t, w_d[k * 128 : (k + 1) * 128, :])
                wsb.append(w_t)
            # full x^T: [KC][4 column groups] tiles of [128, 1024] (2KB lines).
            # Group-wise loads let the h-phase start after the first group.
            msb = [
                mp.tile([128, 4 * SHARD], BF16, name=f"msb{b}", tag=f"msb{b}")
                for b in range(NT // 4)
            ]

            def load_mask(b):
                nc.sync.dma_start(msb[b], m_d[b])

            NG = 4
            GW = N // NG
            xsb = [[None] * NG for _ in range(KC)]
            for g in range(NG):
                for k in range(KC):
                    x_t = cp.tile([128, GW], BF16, name=f"xsb{k}_{g}",
                                  tag=f"xsb{k}_{g}")
                    nc.sync.dma_start(
                        x_t, xt_d[k * 128 : (k + 1) * 128, g * GW : (g + 1) * GW]
                    )
                    xsb[k][g] = x_t
                if g < 2:
                    load_mask(g)
            ident = cp.tile([128, 128], F32, name="ident", tag="ident")
            from concourse.masks import make_identity

            make_identity(nc, ident)
            # warm the ACT exp table while DMAs land
            exp_warm = cp.tile([1, 128], F32, name="exp_warm", tag="exp_warm")
            nc.scalar.activation(exp_warm, ident[0:1, :], act.Exp)
            ones_row_f32 = cp.tile([1, 128], F32, name="ones_row_f32",
                                   tag="ones_row_f32")
            nc.gpsimd.memset(ones_row_f32, 1.0)
            ones_row = cp.tile([1, 128], BF16, name="ones_row", tag="ones_row")
            nc.scalar.copy(ones_row, ones_row_f32)
            ones_col_f32 = cp.tile([128, HEADS], F32, name="ones_col_f32",
                                   tag="ones_col_f32")
            nc.gpsimd.memset(ones_col_f32, 1.0)
            cbias = cp.tile([128, 1], F32, name="cbias", tag="cbias")
            nc.gpsimd.memset(cbias, -CSHIFT)

            # ---------------- phases C+B interleaved.
            # C: es8_b4[j, k*512+i] = exp(0.8 * s[shard_i, k]) broadcast.
            # B: all h-compute. The PE queue runs the h matmuls gated only on
            # DMA; the phase-C broadcast matmuls are interleaved between the
            # first h tiles so PE never stalls on phase-C's ACT copies. The
            # DVE elementwise (phase D) starts as soon as es8_b4 lands.
            acc = [
                psa.tile([HB, SHARD], F32, name=f"acc{k}", tag=f"acc{k}")
                for k in range(HEADS)
            ]
            st_sb = [
                cp.tile([1, SHARD], BF16, name=f"st_sb{k}", tag=f"st_sb{k}")
                for k in range(HEADS)
            ]
            es8_b4 = cp.tile([128, HEADS * SHARD], BF16, name="es8_b4", tag="es8_b4")
            def st_compute(k):
                st_ps = psp.tile([1, SHARD], F32, name="st_ps", tag="pstmp")
                for kc in range(KC):
                    nc.tensor.matmul(
                        st_ps,
                        lhsT=wst[kc][:, k : k + 1],
                        rhs=xst[kc],
                        start=(kc == 0),
                        stop=(kc == KC - 1),
                    )
                nc.scalar.copy(st_sb[k], st_ps)

            def bcast_es8(k):
                sb_ps = psp.tile([128, SHARD], F32, name="sb_ps", tag="pstmp")
                nc.tensor.matmul(
                    sb_ps,
                    lhsT=ones_row,
                    rhs=st_sb[k],
                    start=True,
                    stop=True,
                )
                nc.scalar.activation(
                    es8_b4[:, k * SHARD : (k + 1) * SHARD], sb_ps, act.Exp,
                    scale=1.0 - ALPHA,
                )

            h_sb = []
            et82_sb = []

            def h_tile(nt):
                if nt % 4 == 0 and 2 + nt // 4 < NT // 4:
                    load_mask(2 + nt // 4)
                ph = php.tile([128, WCOLS], F32, name="ph", tag="ph")
                g, r = nt // (GW // 128), nt % (GW // 128)
                for k in range(KC):
                    nc.tensor.matmul(
                        ph,
                        lhsT=xsb[k][g][:, r * 128 : (r + 1) * 128],
                        rhs=wsb[k],
                        start=(k == 0),
                        stop=(k == KC - 1),
                    )
                # per-partition exp scalars: [e^{t-c} (4) | e^{0.2t-c} (4)]
                et82 = hp.tile([128, 2 * HEADS], F32, name=f"et82_{nt}",
                               tag=f"et82_{nt}")
                nc.scalar.activation(et82, ph[:, TCOL:], act.Exp, bias=cbias)
                # pack h into weights layout [h0|1|h1|1|h2|1|h3|1]
                h_t = hp.tile([128, HEADS * HB], BF16,
                              name=f"h_sb{nt}", tag=f"h_sb{nt}")
                nc.gpsimd.tensor_copy(
                    h_t.rearrange("p (h c) -> p h c", c=HB)[
                        :, :, OUT_F : OUT_F + 1
                    ],
                    ones_col_f32.rearrange("p (h c) -> p h c", c=1),
                )
                nc.scalar.copy(
                    h_t.rearrange("p (h c) -> p h c", c=HB)[:, :, :OUT_F],
                    ph[:, : HEADS * OUT_F].rearrange("p (h c) -> p h c", c=OUT_F),
                )
                h_sb.append(h_t)
                et82_sb.append(et82)

            # head-0's s chain first so the DVE can start ASAP, then the other
            # heads' chains interleaved between early h tiles.
            st_compute(0)
            h_tile(0)
            bcast_es8(0)
            st_compute(1)
            h_tile(1)
            bcast_es8(1)
            st_compute(2)
            h_tile(2)
            bcast_es8(2)
            st_compute(3)
            h_tile(3)
            bcast_es8(3)
            for nt in range(HEADS, NT):
                h_tile(nt)

            # ---------------- phase D: elementwise surrogate + attn matmuls
            #   rp_k = max(es8 * e^{t_k - c}, e^{0.2 t_k - c})   (TS, 4x)
            #   at   = rp * m01  (one 2048-wide TT, mask broadcast over heads)
            for jt in range(NT):
                et82 = et82_sb[jt]
                rp = wp.tile([128, HEADS * SHARD], BF16, name="rp", tag="rp")
                for k in range(HEADS):
                    nc.vector.tensor_scalar(
                        rp[:, k * SHARD : (k + 1) * SHARD],
                        es8_b4[:, k * SHARD : (k + 1) * SHARD],
                        et82[:, k : k + 1],
                        et82[:, HEADS + k : HEADS + k + 1],
                        AluOpType.mult,
                        AluOpType.max,
                    )
                at = wp.tile([128, HEADS * SHARD], BF16, name="at", tag="at")
                mview = (
                    msb[jt // 4][:, (jt % 4) * SHARD : (jt % 4 + 1) * SHARD]
                    .rearrange("p (one i) -> p one i", one=1)
                    .broadcast_to((128, HEADS, SHARD))
                )
                nc.vector.tensor_tensor(
                    at.rearrange("p (h i) -> p h i", h=HEADS),
                    rp.rearrange("p (h i) -> p h i", h=HEADS),
                    mview,
                    AluOpType.mult,
                )
                for k in range(HEADS):
                    nc.tensor.matmul(
                        acc[k],
                        lhsT=h_sb[jt][:, k * HB : (k + 1) * HB],
                        rhs=at[:, k * SHARD : (k + 1) * SHARD],
                        start=(jt == 0),
                        stop=(jt == NT - 1),
                    )

            # ---------------- endgame: transpose, normalize, store
            out_sb = [
                ep.tile([128, HEADS * OUT_F], F32, name=f"osb{c}", tag=f"osb{c}",
                        bufs=1)
                for c in range(SHARD // 128)
            ]
            for k in range(HEADS):
                num_sb = ep.tile([HB, SHARD], F32, name="num_sb", tag="num_sb")
                nc.scalar.copy(num_sb, acc[k])
                for c in range(SHARD // 128):
                    tp = psp.tile([128, HB], F32, name="tp", tag="pstmp")
                    nc.tensor.transpose(
                        tp, num_sb[:, c * 128 : (c + 1) * 128], ident[:HB, :HB]
                    )
                    rec = ep.tile([128, 1], F32, name="rec", tag="rec", bufs=4)
                    nc.vector.reciprocal(rec, tp[:, OUT_F : OUT_F + 1])
                    nc.scalar.activation(
                        out_sb[c][:, k * OUT_F : (k + 1) * OUT_F],
                        tp[:, :OUT_F],
                        act.Copy,
                        scale=rec,
                    )
            for c in range(SHARD // 128):
                nc.sync.dma_start(out_d[c * 128 : (c + 1) * 128, :], out_sb[c])

    nc.finalize()
    _PROGRAM = nc
    return nc


# ------------------------------------------------------------------- driver
LAST_RESULT = None


def kernel(x, adj, W, a):
    global LAST_RESULT
    x = np.asarray(x, dtype=np.float32)
    adj = np.asarray(adj)
    W = np.asarray(W, dtype=np.float32)
    a = np.asarray(a, dtype=np.float32)

    # ---- host-side layout prep (sharding + transposes, no math on the data
    # beyond folding the tiny attention vectors into W)
    a_src = a[:OUT_F, 0]
    a_dst = a[OUT_F:, 0]
    Wh = W.reshape(IN_F, HEADS, OUT_F)
    Ws = np.einsum("fhc,c->fh", Wh, a_src)       # [IN_F, HEADS]
    Wt = np.einsum("fhc,c->fh", Wh, a_dst)
    wext = np.ascontiguousarray(
        np.concatenate([W, Ws, Wt, ALPHA * Wt], axis=1)
    ).astype(ml_dtypes.bfloat16)                 # [512, 268]

    xT = np.ascontiguousarray(x.T).astype(ml_dtypes.bfloat16)   # [512, 4096]
    m01 = (adj.T != 0).astype(ml_dtypes.bfloat16)               # [4096, 4096]

    in_maps = []
    for c in range(NCORES):
        sl = slice(c * SHARD, (c + 1) * SHARD)
        # pre-tile the mask: [b, p, q*i] with row (q, i) contiguous (4KB lines)
        mtb = np.ascontiguousarray(
            m01[:, sl].reshape(NT // 4, 4, 128, SHARD)
            .transpose(0, 2, 1, 3)
            .reshape(NT // 4, 128, 4 * SHARD)
        )
        in_maps.append(
            {
                "xT": xT,
                "xsT": np.ascontiguousarray(xT[:, sl]),
                "wext": wext,
                "maskTb": mtb,
            }
        )

    nc = _build_program()
    res = bass_utils.run_bass_kernel_spmd(
        nc,
        in_maps,
        core_ids=list(range(NCORES)),
        trace=os.environ.get("GAT_TRACE", "0") == "1",
    )
    LAST_RESULT = res
    out = np.concatenate([r["out"] for r in res.results], axis=0)
    return out.astype(np.float32)


# revision 30
# speedup vs baseline: 1.0023x; 1.0023x over previous
"""GAT layer (dense formulation) on 8 Trainium2 NeuronCores.

Computation (N=4096 nodes, IN_F=512, OUT_F=64, HEADS=4):
    h = (x @ W).reshape(N, H, F)
    s = h . a_src ; t = h . a_dst            (per node, per head)
    e[i,j,k] = leaky_relu(s[i,k] + t[j,k])   masked by adj[i,j]
    attn = softmax_j(e) ; out = attn @ h

Sharding: output rows i (nodes) are sharded 512/core across 8 cores.
Each core computes the full h = x @ W redundantly (cheap), then handles
its own 512 i-rows: logits laid out [j=partitions, i=free] so that the
softmax contraction over j runs on the PE as  [h_k | 1].T @ at_tile,
with the ones-column producing the softmax denominator for free.

V4 factorization — no per-element exp at all. Using
    lrelu(v) = 0.2 v + 0.8 relu(v),   v = s_i + t_j:
    exp(lrelu(v)) = e^{0.2 s} * e^{0.2 t} * max(e^{0.8 v}, 1)
The e^{0.2 s_i} factor is constant along the softmax axis j, so it
cancels between numerator and denominator and is simply dropped. With a
global shift e^{-c} (also cancels) the attention surrogate is
    at[j,i] = max( e^{0.8 s_i} * e^{t_j - c},  e^{0.2 t_j - c} ) * m01
= ONE tensor_scalar (mult,max — 4x bf16) + ONE tensor_tensor mult
(2x bf16) per head-tile on the DVE, per-partition exp scalars coming
from a single tiny [128, 8] ACT exp per tile. The t and 0.2t columns
are produced by the h-matmul itself via extra folded W columns.
"""

import os

import numpy as np
import ml_dtypes

import concourse.bass as bass
import concourse.mybir as mybir
import concourse.tile as tile
from concourse import bacc, bass_utils
from concourse._compat import get_trn_type
from concourse.alu_op_type import AluOpType

# ---------------------------------------------------------------- constants
N = 4096
IN_F = 512
OUT_F = 64
HEADS = 4
ALPHA = 0.2
NCORES = 8
SHARD = N // NCORES            # 512 output rows per core
NT = N // 128                  # 32 j-tiles (and n-tiles)
KC = IN_F // 128               # 4 contraction chunks
# W_ext = [W(256) | Ws(4) | Wt(4) | 0.2*Wt(4)]
WCOLS = HEADS * OUT_F + 3 * HEADS   # 268
TCOL = HEADS * OUT_F + HEADS        # 260: start of the [t | 0.2t] block
HB = OUT_F + 1                 # 65 = per-head [h_k | ones] weight block
DELAY = 2                      # attn matmuls trail h-compute by this many tiles
CSHIFT = 3.0                   # global exp shift (cancels in softmax)

F32 = mybir.dt.float32
BF16 = mybir.dt.bfloat16

# ------------------------------------------------------------- bass program
_PROGRAM = None


def _build_program():
    """One SPMD program; per-core behavior differs only through input data."""
    global _PROGRAM
    if _PROGRAM is not None:
        return _PROGRAM

    nc = bacc.Bacc(get_trn_type() or "TRN2", target_bir_lowering=False)
    act = mybir.ActivationFunctionType

    # x^T, full: xT[f, n] = x[n, f], bf16
    xt_d = nc.dram_tensor("xT", [IN_F, N], BF16, kind="ExternalInput")
    # x-shard transposed: xsT[f, i] = x[shard_start + i, f], bf16
    xs_d = nc.dram_tensor("xsT", [IN_F, SHARD], BF16, kind="ExternalInput")
    w_d = nc.dram_tensor("wext", [IN_F, WCOLS], BF16, kind="ExternalInput")
    # multiplicative mask {0,1}, pre-tiled on host: m01[b, p, q*i] =
    #   (adj.T[b*512 + q*128 + p, shard_i] != 0)   (bf16; 4KB DMA lines)
    m_d = nc.dram_tensor("maskTb", [NT // 4, 128, 4 * SHARD], BF16,
                         kind="ExternalInput")
    out_d = nc.dram_tensor("out", [SHARD, HEADS * OUT_F], F32, kind="ExternalOutput")

    with tile.TileContext(nc) as tc:
        with (
            tc.tile_pool(name="const", bufs=1) as cp,
            tc.tile_pool(name="hpool", bufs=1) as hp,
            tc.tile_pool(name="mpool", bufs=1) as mp,
            tc.tile_pool(name="work", bufs=6) as wp,
            tc.tile_pool(name="endp", bufs=2) as ep,
            tc.tile_pool(name="ps", bufs=2, space="PSUM") as psp,
            tc.tile_pool(name="ph", bufs=2, space="PSUM") as php,
            tc.tile_pool(name="psacc", bufs=1, space="PSUM") as psa,
        ):
            # ---------------- phase A: constants in (one DMA per tensor —
            # successive dma_start instructions serialize at ~0.7us each on
            # the sync queue, so batch via multi-level access patterns)
            xst_b = cp.tile([128, KC * SHARD], BF16, name="xst", tag="xst")
            nc.sync.dma_start(
                xst_b.rearrange("p (k i) -> p k i", k=KC),
                xs_d.rearrange("(k p) i -> p k i", p=128),
            )
            xst = [xst_b[:, k * SHARD : (k + 1) * SHARD] for k in range(KC)]
            wst_b = cp.tile([128, KC * 3 * HEADS], BF16, name="wst", tag="wst")
            nc.sync.dma_start(
                wst_b.rearrange("p (k c) -> p k c", k=KC),
                w_d[:, HEADS * OUT_F :].rearrange("(k p) c -> p k c", p=128),
            )
            wst = [wst_b[:, k * 3 * HEADS : (k + 1) * 3 * HEADS] for k in range(KC)]
            wsb_b = cp.tile([128, KC * WCOLS], BF16, name="wsb", tag="wsb")
            nc.sync.dma_start(
                wsb_b.rearrange("p (k c) -> p k c", k=KC),
                w_d.rearrange("(k p) c -> p k c", p=128),
            )
            wsb = [wsb_b[:, k * WCOLS : (k + 1) * WCOLS] for k in range(KC)]
            # full x^T: [KC][4 column groups] tiles of [128, 1024] (2KB lines).
            # Group-wise loads let the h-phase start after the first group.
            msb = [
                mp.tile([128, 4 * SHARD], BF16, name=f"msb{b}", tag=f"msb{b}")
                for b in range(NT // 4)
            ]

            def load_mask(b):
                nc.sync.dma_start(msb[b], m_d[b])

            NG = 4
            GW = N // NG
            xsb_g = []
            for g in range(NG):
                x_t = cp.tile([128, KC * GW], BF16, name=f"xsb_{g}",
                              tag=f"xsb_{g}")
                nc.sync.dma_start(
                    x_t.rearrange("p (k n) -> p k n", k=KC),
                    xt_d[:, g * GW : (g + 1) * GW].rearrange(
                        "(k p) n -> p k n", p=128
                    ),
                )
                xsb_g.append(x_t)
                if g < 2:
                    load_mask(g)
            xsb = [
                [xsb_g[g][:, k * GW : (k + 1) * GW] for g in range(NG)]
                for k in range(KC)
            ]
            ident = cp.tile([128, 128], F32, name="ident", tag="ident")
            from concourse.masks import make_identity

            make_identity(nc, ident)
            # warm the ACT exp table while DMAs land
            exp_warm = cp.tile([1, 128], F32, name="exp_warm", tag="exp_warm")
            nc.scalar.activation(exp_warm, ident[0:1, :], act.Exp)
            ones_row_f32 = cp.tile([1, 128], F32, name="ones_row_f32",
                                   tag="ones_row_f32")
            nc.gpsimd.memset(ones_row_f32, 1.0)
            ones_row = cp.tile([1, 128], BF16, name="ones_row", tag="ones_row")
            nc.scalar.copy(ones_row, ones_row_f32)
            ones_col_f32 = cp.tile([128, HEADS], F32, name="ones_col_f32",
                                   tag="ones_col_f32")
            nc.gpsimd.memset(ones_col_f32, 1.0)
            cbias = cp.tile([128, 1], F32, name="cbias", tag="cbias")
            nc.gpsimd.memset(cbias, -CSHIFT)

            # ---------------- phases C+B interleaved.
            # C: es8_b4[j, k*512+i] = exp(0.8 * s[shard_i, k]) broadcast.
            # B: all h-compute. The PE queue runs the h matmuls gated only on
            # DMA; the phase-C broadcast matmuls are interleaved between the
            # first h tiles so PE never stalls on phase-C's ACT copies. The
            # DVE elementwise (phase D) starts as soon as es8_b4 lands.
            acc = [
                psa.tile([HB, SHARD], F32, name=f"acc{k}", tag=f"acc{k}")
                for k in range(HEADS)
            ]
            st_sb = [
                cp.tile([1, SHARD], BF16, name=f"st_sb{k}", tag=f"st_sb{k}")
                for k in range(HEADS)
            ]
            es8_b4 = cp.tile([128, HEADS * SHARD], BF16, name="es8_b4", tag="es8_b4")
            def st_compute(k):
                st_ps = psp.tile([1, SHARD], F32, name="st_ps", tag="pstmp")
                for kc in range(KC):
                    nc.tensor.matmul(
                        st_ps,
                        lhsT=wst[kc][:, k : k + 1],
                        rhs=xst[kc],
                        start=(kc == 0),
                        stop=(kc == KC - 1),
                    )
                nc.scalar.copy(st_sb[k], st_ps)

            def bcast_es8(k):
                sb_ps = psp.tile([128, SHARD], F32, name="sb_ps", tag="pstmp")
                nc.tensor.matmul(
                    sb_ps,
                    lhsT=ones_row,
                    rhs=st_sb[k],
                    start=True,
                    stop=True,
                )
                nc.scalar.activation(
                    es8_b4[:, k * SHARD : (k + 1) * SHARD], sb_ps, act.Exp,
                    scale=1.0 - ALPHA,
                )

            h_sb = []
            et82_sb = []

            def h_tile(nt):
                if nt % 4 == 0 and 2 + nt // 4 < NT // 4:
                    load_mask(2 + nt // 4)
                ph = php.tile([128, WCOLS], F32, name="ph", tag="ph")
                g, r = nt // (GW // 128), nt % (GW // 128)
                for k in range(KC):
                    nc.tensor.matmul(
                        ph,
                        lhsT=xsb[k][g][:, r * 128 : (r + 1) * 128],
                        rhs=wsb[k],
                        start=(k == 0),
                        stop=(k == KC - 1),
                    )
                # per-partition exp scalars: [e^{t-c} (4) | e^{0.2t-c} (4)]
                et82 = hp.tile([128, 2 * HEADS], F32, name=f"et82_{nt}",
                               tag=f"et82_{nt}")
                nc.scalar.activation(et82, ph[:, TCOL:], act.Exp, bias=cbias)
                # pack h into weights layout [h0|1|h1|1|h2|1|h3|1]
                h_t = hp.tile([128, HEADS * HB], BF16,
                              name=f"h_sb{nt}", tag=f"h_sb{nt}")
                nc.gpsimd.tensor_copy(
                    h_t.rearrange("p (h c) -> p h c", c=HB)[
                        :, :, OUT_F : OUT_F + 1
                    ],
                    ones_col_f32.rearrange("p (h c) -> p h c", c=1),
                )
                nc.scalar.copy(
                    h_t.rearrange("p (h c) -> p h c", c=HB)[:, :, :OUT_F],
                    ph[:, : HEADS * OUT_F].rearrange("p (h c) -> p h c", c=OUT_F),
                )
                h_sb.append(h_t)
                et82_sb.append(et82)

            # head-0's s chain first so the DVE can start ASAP, then the other
            # heads' chains interleaved between early h tiles.
            st_compute(0)
            h_tile(0)
            bcast_es8(0)
            st_compute(1)
            h_tile(1)
            bcast_es8(1)
            st_compute(2)
            h_tile(2)
            bcast_es8(2)
            st_compute(3)
            h_tile(3)
            bcast_es8(3)
            for nt in range(HEADS, NT):
                h_tile(nt)

            # ---------------- phase D: elementwise surrogate + attn matmuls
            #   rp_k = max(es8 * e^{t_k - c}, e^{0.2 t_k - c})   (TS, 4x)
            #   at   = rp * m01  (one 2048-wide TT, mask broadcast over heads)
            for jt in range(NT):
                et82 = et82_sb[jt]
                rp = wp.tile([128, HEADS * SHARD], BF16, name="rp", tag="rp")
                for k in range(HEADS):
                    nc.vector.tensor_scalar(
                        rp[:, k * SHARD : (k + 1) * SHARD],
                        es8_b4[:, k * SHARD : (k + 1) * SHARD],
                        et82[:, k : k + 1],
                        et82[:, HEADS + k : HEADS + k + 1],
                        AluOpType.mult,
                        AluOpType.max,
                    )
                at = wp.tile([128, HEADS * SHARD], BF16, name="at", tag="at")
                mview = (
                    msb[jt // 4][:, (jt % 4) * SHARD : (jt % 4 + 1) * SHARD]
                    .rearrange("p (one i) -> p one i", one=1)
                    .broadcast_to((128, HEADS, SHARD))
                )
                nc.vector.tensor_tensor(
                    at.rearrange("p (h i) -> p h i", h=HEADS),
                    rp.rearrange("p (h i) -> p h i", h=HEADS),
                    mview,
                    AluOpType.mult,
                )
                for k in range(HEADS):
                    nc.tensor.matmul(
                        acc[k],
                        lhsT=h_sb[jt][:, k * HB : (k + 1) * HB],
                        rhs=at[:, k * SHARD : (k + 1) * SHARD],
                        start=(jt == 0),
                        stop=(jt == NT - 1),
                    )

            # ---------------- endgame: transpose, normalize, store
            out_sb = [
                ep.tile([128, HEADS * OUT_F], F32, name=f"osb{c}", tag=f"osb{c}",
                        bufs=1)
                for c in range(SHARD // 128)
            ]
            for k in range(HEADS):
                num_sb = ep.tile([HB, SHARD], F32, name="num_sb", tag="num_sb")
                nc.scalar.copy(num_sb, acc[k])
                for c in range(SHARD // 128):
                    tp = psp.tile([128, HB], F32, name="tp", tag="pstmp")
                    nc.tensor.transpose(
                        tp, num_sb[:, c * 128 : (c + 1) * 128], ident[:HB, :HB]
                    )
                    rec = ep.tile([128, 1], F32, name="rec", tag="rec", bufs=4)
                    nc.vector.reciprocal(rec, tp[:, OUT_F : OUT_F + 1])
                    nc.scalar.activation(
                        out_sb[c][:, k * OUT_F : (k + 1) * OUT_F],
                        tp[:, :OUT_F],
                        act.Copy,
                        scale=rec,
                    )
            for c in range(SHARD // 128):
                nc.sync.dma_start(out_d[c * 128 : (c + 1) * 128, :], out_sb[c])

    nc.finalize()
    _PROGRAM = nc
    return nc


# ------------------------------------------------------------------- driver
LAST_RESULT = None


def kernel(x, adj, W, a):
    global LAST_RESULT
    x = np.asarray(x, dtype=np.float32)
    adj = np.asarray(adj)
    W = np.asarray(W, dtype=np.float32)
    a = np.asarray(a, dtype=np.float32)

    # ---- host-side layout prep (sharding + transposes, no math on the data
    # beyond folding the tiny attention vectors into W)
    a_src = a[:OUT_F, 0]
    a_dst = a[OUT_F:, 0]
    Wh = W.reshape(IN_F, HEADS, OUT_F)
    Ws = np.einsum("fhc,c->fh", Wh, a_src)       # [IN_F, HEADS]
    Wt = np.einsum("fhc,c->fh", Wh, a_dst)
    wext = np.ascontiguousarray(
        np.concatenate([W, Ws, Wt, ALPHA * Wt], axis=1)
    ).astype(ml_dtypes.bfloat16)                 # [512, 268]

    xT = np.ascontiguousarray(x.T).astype(ml_dtypes.bfloat16)   # [512, 4096]
    m01 = (adj.T != 0).astype(ml_dtypes.bfloat16)               # [4096, 4096]

    in_maps = []
    for c in range(NCORES):
        sl = slice(c * SHARD, (c + 1) * SHARD)
        # pre-tile the mask: [b, p, q*i] with row (q, i) contiguous (4KB lines)
        mtb = np.ascontiguousarray(
            m01[:, sl].reshape(NT // 4, 4, 128, SHARD)
            .transpose(0, 2, 1, 3)
            .reshape(NT // 4, 128, 4 * SHARD)
        )
        in_maps.append(
            {
                "xT": xT,
                "xsT": np.ascontiguousarray(xT[:, sl]),
                "wext": wext,
                "maskTb": mtb,
            }
        )

    nc = _build_program()
    res = bass_utils.run_bass_kernel_spmd(
        nc,
        in_maps,
        core_ids=list(range(NCORES)),
        trace=os.environ.get("GAT_TRACE", "0") == "1",
    )
    LAST_RESULT = res
    out = np.concatenate([r["out"] for r in res.results], axis=0)
    return out.astype(np.float32)
